# revision 12
# baseline (speedup 1.0000x reference)
"""Trainium2 Bass kernel for an 11-stage butterfly linear layer + bias.

out[r, :] = B @ x[r, :] + bias, B = 11 butterfly stages (strides 1..1024).
Positions p = b*128 + t*8 + w8 (b,t in [0,16), w8 in [0,8)); w = t*8 + w8.
B = C * D: D = diag(D_0..D_15) (stages 0-6, dense 128x128 per block b),
C = per-w 16x16 C_w mixing the block index b (stages 7-10).

Per core (2048 batch rows, host-transposed position-major bf16 input):
  Phase 1, per 512-col chunk: DMA in -> 16 MM1 (D_b) -> drain (vector/scalar,
  strided to put (f*16+b) order in Mid) -> ONE XBAR dma-transpose per chunk:
  mid2[p=(f7lo,b), j=(Q,f7hi), l=(t,w8)] = Mid[(t,w8), j*128+p].
  Phase 2: 128 MM2 (one per w, stationary = C_w x I_8 block-diag over f7lo,
  contracting partitions (f7lo, b); moving = mid2[:, :, w], full 2048-batch
  stream so each stationary loads once) -> drain -> DMA out in 32-w groups
  (64KB-contiguous descriptors). Host un-permutes output + adds bias (fp32).

HBM: 8.4 MB in + 8.4 MB out per core (bf16); corner turn rides the XBAR
(no SBUF->SBUF DMA-engine traffic, ~1 trigger/chunk).
"""

import sys

import numpy as np

sys.path.insert(0, "/opt/trn_rl_repo")

import concourse.bass as bass  # noqa: E402
import concourse.mybir as mybir  # noqa: E402
import concourse.tile as tile  # noqa: E402
from concourse import bacc  # noqa: E402
from concourse.bass import ds, ts  # noqa: E402
from concourse.bass_utils import run_bass_kernel_spmd  # noqa: E402

N = 2048
LOG_N = 11
NCORES = 8
BATCH = 16384
BPC = BATCH // NCORES
P = 128
NB = 16
CH = 512
NCH = BPC // CH
NJ = (BPC // P) * 16  # total (Q, f7hi) column-groups = 256 per core
NJC = CH // P * 16  # j-groups per chunk = 64
WGRP = 32           # w's per output DMA group

PROFILE = False
LAST_RESULTS = None

_NC_CACHE = {}


def _emit_body(ctx, tc, aps):
    nc = tc.nc
    x_ap, w1_ap, c3_ap, out_ap = aps
    f32 = mybir.dt.float32
    bf16 = mybir.dt.bfloat16

    const = ctx.enter_context(tc.tile_pool(name="const", bufs=1))
    W1 = const.tile([P, NB * P], bf16)
    C3 = const.tile([P, P * P], bf16)
    nc.sync.dma_start(W1[:], w1_ap)
    nc.sync.dma_start(C3[:], c3_ap)

    xpool = ctx.enter_context(tc.tile_pool(name="xin", bufs=2))
    mpool = ctx.enter_context(tc.tile_pool(name="mid", bufs=2))
    m2pool = ctx.enter_context(tc.tile_pool(name="mid2", bufs=1))
    opool = ctx.enter_context(tc.tile_pool(name="oout", bufs=2))
    ps1 = ctx.enter_context(tc.tile_pool(name="ps1", bufs=3, space="PSUM"))
    ps2 = ctx.enter_context(tc.tile_pool(name="ps2", bufs=4, space="PSUM"))

    x_src = x_ap.rearrange("(b w) f -> w b f", b=NB)

    mid2 = m2pool.tile([P, NJ * P], bf16, name="mid2", tag="mid2")

    xts = [xpool.tile([P, NB * CH], bf16, name=f"xt_{c}", tag="xt") for c in range(NCH)]

    # ---- Phase 1: MM1 + XBAR corner turn, per chunk ----
    for c in range(NCH):
        if c == 0:
            nc.sync.dma_start(
                xts[0][:].rearrange("p (b f) -> p b f", b=NB), x_src[:, :, ds(0, CH)]
            )
        Mid = mpool.tile([P, CH * NB], bf16, name=f"mid_{c}", tag="mid")
        mid_fb = Mid[:].rearrange("p (f b) -> p f b", b=NB)
        for b in range(NB):
            py = ps1.tile([P, CH], f32, name=f"py_{c}_{b}", tag="py")
            nc.tensor.matmul(
                py[:], W1[:, ts(b, P)], xts[c][:, ts(b, CH)], start=True, stop=True
            )
            if b % 2 == 0:
                nc.vector.tensor_copy(mid_fb[:, :, b], py[:])
            else:
                nc.scalar.copy(mid_fb[:, :, b], py[:])

        if c + 1 < NCH:
            nc.sync.dma_start(
                xts[c + 1][:].rearrange("p (b f) -> p b f", b=NB),
                x_src[:, :, ds((c + 1) * CH, CH)],
            )

        # mid2[p=(f7lo,b), j=(q4,f7hi), l=(t,w8)] = Mid[l, j*128 + p]
        nc.sync.dma_start_transpose(
            mid2[:, ds(c * NJC * P, NJC * P)].rearrange("p (j l) -> p j l", j=NJC),
            Mid[:],
        )

    # ---- Phase 2: 128 MM2 (one per w), full-batch streams ----
    mid2_lw = mid2[:].rearrange("p (j l) -> p l j", j=NJ, l=P)
    og = None
    for w in range(P):
        g, lw = divmod(w, WGRP)
        if lw == 0:
            og = opool.tile([P, WGRP * NJ], bf16, name=f"o_{g}", tag="o")
        pz = ps2.tile([P, NJ], f32, name=f"pz_{w}", tag="pz")
        nc.tensor.matmul(
            pz[:], C3[:, ts(w, P)], mid2_lw[:, w, :], start=True, stop=True
        )
        if w % 2 == 0:
            nc.vector.tensor_copy(og[:, ts(lw, NJ)], pz[:])
        else:
            nc.scalar.copy(og[:, ts(lw, NJ)], pz[:])
        if lw == WGRP - 1:
            nc.sync.dma_start(out_ap[:, ds(g * WGRP * NJ, WGRP * NJ)], og[:])


def build_nc():
    nc = bacc.Bacc(
        "TRN2",
        target_bir_lowering=False,
        debug=False,
        num_devices=NCORES,
    )
    bf16 = mybir.dt.bfloat16
    x_ap = nc.dram_tensor("xt", [N, BPC], bf16, kind="ExternalInput").ap()
    w1_ap = nc.dram_tensor("w1", [P, NB * P], bf16, kind="ExternalInput").ap()
    c3_ap = nc.dram_tensor("c3", [P, P * P], bf16, kind="ExternalInput").ap()
    out_ap = nc.dram_tensor("out", [P, P * NJ], bf16, kind="ExternalOutput").ap()

    from contextlib import ExitStack

    with tile.TileContext(nc) as tc:
        with ExitStack() as ctx:
            _emit_body(ctx, tc, (x_ap, w1_ap, c3_ap, out_ap))
    nc.compile()
    return nc


def _butterfly_apply(tw, X, idx_lo, idx_hi):
    out = X
    for idx in range(idx_lo, idx_hi):
        s = 1 << idx
        g = N // (2 * s)
        T = tw[idx].reshape(g, s, 2, 2)
        xr = out.reshape(-1, g, 2, s)
        out = np.einsum("gsij,bgjs->bgis", T, xr).reshape(-1, N)
    return out


def host_weights(twiddle):
    import ml_dtypes

    tw = np.asarray(twiddle, dtype=np.float64)[0, 0]
    eye = np.eye(N, dtype=np.float64)
    R1 = _butterfly_apply(tw, eye, 0, 7)      # R1[p_in, p_out] = D[p_out, p_in]
    R2 = _butterfly_apply(tw, eye, 7, LOG_N)  # R2[p_in, p_out] = C[p_out, p_in]

    w1 = np.concatenate(
        [R1[b * P : (b + 1) * P, b * P : (b + 1) * P] for b in range(NB)], axis=1
    )
    bidx = np.arange(NB)
    c3 = np.zeros((P, P * P))
    for w in range(P):
        M = R2[np.ix_(bidx * P + w, bidx * P + w)]  # [b_in, b_out]
        for lo in range(8):
            c3[lo * 16 : lo * 16 + 16, w * P + lo * 16 : w * P + lo * 16 + 16] = M
    return (
        np.ascontiguousarray(w1.astype(ml_dtypes.bfloat16)),
        np.ascontiguousarray(c3.astype(ml_dtypes.bfloat16)),
    )


def kernel(x, twiddle, bias):
    global LAST_RESULTS
    import ml_dtypes

    x = np.asarray(x)
    assert x.shape == (BATCH, N), x.shape
    xbf = x.astype(ml_dtypes.bfloat16)

    if "nc" not in _NC_CACHE:
        _NC_CACHE["nc"] = build_nc()
    nc = _NC_CACHE["nc"]

    w1, c3 = host_weights(twiddle)
    in_maps = []
    for c in range(NCORES):
        xt = np.ascontiguousarray(xbf[c * BPC : (c + 1) * BPC].T)
        in_maps.append({"xt": xt, "w1": w1, "c3": c3})
    res = run_bass_kernel_spmd(nc, in_maps, core_ids=list(range(NCORES)), trace=PROFILE)
    LAST_RESULTS = res

    bias32 = np.asarray(bias, dtype=np.float32)[None, :]
    outs = []
    for c in range(NCORES):
        od = np.asarray(res.results[c]["out"]).astype(np.float32)
        # od[p=(f7lo,b'), w=(t,w8), j=(Q,f7hi)] -> out[f=(Q,f7hi,f7lo), p'=(b',t,w8)]
        o = od.reshape(8, 16, 16, 8, NJ // 16, 16)  # f7lo, b', t, w8, Q, f7hi
        outs.append(
            np.transpose(o, (4, 5, 0, 1, 2, 3)).reshape(BPC, N) + bias32
        )
    return np.concatenate(outs, axis=0)
